# revision 4
# baseline (speedup 1.0000x reference)
"""Trainium2 Bass kernel for nn_BoWDecoder (cross-entropy loss of a 2-layer
bag-of-words decoder head), tensor-parallel over the vocab axis on 8 cores.

Computation (full, fp32):
    h      = gelu(z @ W0 + b0)            # [32, 1024]
    logits = h @ W1 + b1                  # [32, 50257]
    loss   = mean over (32, 128) of -log_softmax(logits)[b, labels[b, s]]

Distribution: W1/b1 sharded along vocab (6400 padded columns per core);
z/W0/b0 replicated (h is recomputed on every core, it is tiny); the label
gather is expressed as a per-row dot product with a count matrix C built
from labels on the host (pure integer bookkeeping), so each core reduces
its vocab shard to 16 per-tile partial stats:
    m[b,t]  = max_v logits[b, v in tile t]          (shipped negated)
    s[b,t]  = sum_v exp(logits[b,v] - m[b,t])
    T[b,t]  = sum_v logits[b,v] * C[b,v]
The host merges the 8*16 partial (m, s) pairs per batch row into the exact
log-sum-exp and assembles the scalar loss (the "unshard" step).

Vocab padding uses W1 columns = 0 and b1 = -1e30 so padded logits vanish
from the softmax exactly.
"""

import numpy as np

import concourse.bacc as bacc
import concourse.mybir as mybir
import concourse.tile as tile
from concourse.bass import ds
from concourse.bass_utils import run_bass_kernel_spmd
from concourse.masks import make_identity

VOCAB = 50257
D = 1024
B = 32
SEQ = 128
N_CORES = 8
VS = 6400          # padded vocab columns per core  (8 * 6400 = 51200)
NT = 400           # vocab tile size (fp32r needs moving dim >= 256)
NTILES = VS // NT  # 16
KC = D // 128      # 8 contraction chunks of 128
PAD_NEG = -1.0e30

_COMPILED_NC = None


def _build(sim_safe=False):
    f32 = mybir.dt.float32
    f32r = mybir.dt.float32r
    FT = mybir.ActivationFunctionType

    nc = bacc.Bacc("TRN2", target_bir_lowering=False, debug=False,
                   num_devices=N_CORES)

    zT_d = nc.dram_tensor("zT", [D, B], f32, kind="ExternalInput")
    w0_d = nc.dram_tensor("w0", [D, D], f32, kind="ExternalInput")
    b0_d = nc.dram_tensor("b0", [1, D], f32, kind="ExternalInput")
    w1_d = nc.dram_tensor("w1", [D, VS], f32, kind="ExternalInput")
    b1_d = nc.dram_tensor("b1", [1, VS], f32, kind="ExternalInput")
    cnt_d = nc.dram_tensor("cnt", [B, VS], f32, kind="ExternalInput")
    nm_d = nc.dram_tensor("negmax_out", [B, NTILES], f32, kind="ExternalOutput")
    s_d = nc.dram_tensor("sum_out", [B, NTILES], f32, kind="ExternalOutput")
    t_d = nc.dram_tensor("tsum_out", [B, NTILES], f32, kind="ExternalOutput")

    with tile.TileContext(nc) as tc:
        with (
            tc.tile_pool(name="const", bufs=1) as const,
            tc.tile_pool(name="w1pool", bufs=3) as w1pool,
            tc.tile_pool(name="work", bufs=2) as work,
            tc.tile_pool(name="psl_pool", bufs=4, space="PSUM") as psl_pool,
            tc.tile_pool(name="psh_pool", bufs=2, space="PSUM") as psh_pool,
        ):
            # ---- resident inputs ----
            zT = const.tile([128, KC, B], f32r)
            nc.sync.dma_start(
                zT[:], zT_d.ap().rearrange("(c p) b -> p c b", p=128).bitcast(f32r))
            w0 = const.tile([128, KC, D], f32r)
            nc.sync.dma_start(
                w0[:], w0_d.ap().rearrange("(c p) n -> p c n", p=128).bitcast(f32r))
            b0sb = const.tile([1, D], f32r)
            nc.sync.dma_start(b0sb[:], b0_d.ap().bitcast(f32r))
            b1sb = const.tile([1, VS], f32r)
            nc.sync.dma_start(b1sb[:], b1_d.ap().bitcast(f32r))
            cnt = const.tile([B, VS], f32)
            nc.sync.dma_start(cnt[:], cnt_d[:])
            ones_f32 = const.tile([1, B], f32)
            nc.vector.memset(ones_f32[:], 1.0)
            ones = const.tile([1, B], f32r)
            nc.scalar.copy(ones[:], ones_f32[:])
            ident = const.tile([B, B], f32)
            make_identity(nc, ident[:])

            # ---- h = gelu(z @ W0 + b0), then transpose to hT [K, B] ----
            h_sb = const.tile([B, D], f32)
            for j in range(2):
                psh = psh_pool.tile([B, 512], f32, tag="psh")
                for k in range(KC):
                    nc.tensor.matmul(psh[:], zT[:, k, :],
                                     w0[:, k, ds(512 * j, 512)],
                                     start=(k == 0), stop=False)
                nc.tensor.matmul(psh[:], ones[:], b0sb[:, ds(512 * j, 512)],
                                 start=False, stop=True)
                nc.scalar.activation(h_sb[:, ds(512 * j, 512)], psh[:],
                                     FT.Identity if sim_safe else FT.Gelu)

            hT = const.tile([128, KC, B], f32r)
            for c in range(KC):
                pst = psh_pool.tile([128, B], f32, tag="pst")
                nc.tensor.transpose(pst[:], h_sb[:, ds(128 * c, 128)], ident[:])
                # rounding copy fp32 -> fp32r for the next matmul's lhsT
                nc.scalar.copy(hT[:, c, :], pst[:])

            # ---- logits tiles: matmul + streaming softmax stats ----
            negmax = const.tile([B, NTILES], f32)
            s_sb = const.tile([B, NTILES], f32)
            t_sb = const.tile([B, NTILES], f32)
            for t in range(NTILES):
                w1t = w1pool.tile([128, KC, NT], f32r, tag="w1t")
                nc.sync.dma_start(
                    w1t[:],
                    w1_d.ap()[:, ds(NT * t, NT)]
                    .rearrange("(c p) n -> p c n", p=128).bitcast(f32r))
                psl = psl_pool.tile([B, NT], f32, tag="psl")
                for k in range(KC):
                    nc.tensor.matmul(psl[:], hT[:, k, :], w1t[:, k, :],
                                     start=(k == 0), stop=False)
                nc.tensor.matmul(psl[:], ones[:], b1sb[:, ds(NT * t, NT)],
                                 start=False, stop=True)

                nc.vector.tensor_reduce(negmax[:, t:t + 1], psl[:],
                                        axis=mybir.AxisListType.X,
                                        op=mybir.AluOpType.max, negate=True)
                scr = work.tile([B, NT], f32, tag="scr")
                nc.scalar.activation(scr[:], psl[:], FT.Exp,
                                     bias=negmax[:, t:t + 1],
                                     accum_out=s_sb[:, t:t + 1])
                scr2 = work.tile([B, NT], f32, tag="scr2")
                nc.vector.tensor_mul(scr2[:], psl[:], cnt[:, ds(NT * t, NT)])
                nc.vector.tensor_reduce(t_sb[:, t:t + 1], scr2[:],
                                        axis=mybir.AxisListType.X,
                                        op=mybir.AluOpType.add)

            nc.sync.dma_start(nm_d[:], negmax[:])
            nc.sync.dma_start(s_d[:], s_sb[:])
            nc.sync.dma_start(t_d[:], t_sb[:])

    nc.compile()
    return nc


def _get_nc():
    global _COMPILED_NC
    if _COMPILED_NC is None:
        _COMPILED_NC = _build()
    return _COMPILED_NC


def _make_in_maps(z, labels, W0, b0, W1, b1):
    z = np.asarray(z, dtype=np.float32)
    W0 = np.asarray(W0, dtype=np.float32)
    b0 = np.asarray(b0, dtype=np.float32)
    W1 = np.asarray(W1, dtype=np.float32)
    b1 = np.asarray(b1, dtype=np.float32)
    labels = np.asarray(labels)

    zT = np.ascontiguousarray(z.T)
    b0r = np.ascontiguousarray(b0.reshape(1, D))

    # label counts per (row, vocab) — pure re-encoding of labels
    cnt_full = np.zeros((B, N_CORES * VS), dtype=np.float32)
    np.add.at(cnt_full, (np.arange(B)[:, None], labels.astype(np.int64)), 1.0)

    in_maps = []
    for c in range(N_CORES):
        lo = c * VS
        hi = min(lo + VS, VOCAB)
        nreal = max(0, hi - lo)
        w1s = np.zeros((D, VS), dtype=np.float32)
        b1s = np.full((1, VS), PAD_NEG, dtype=np.float32)
        if nreal > 0:
            w1s[:, :nreal] = W1[:, lo:hi]
            b1s[0, :nreal] = b1[lo:hi]
        in_maps.append({
            "zT": zT,
            "w0": W0,
            "b0": b0r,
            "w1": w1s,
            "b1": b1s,
            "cnt": np.ascontiguousarray(cnt_full[:, lo:lo + VS]),
        })
    return in_maps


def _combine(results):
    # per-core [B, NTILES] partials -> [B, N_CORES*NTILES]
    m = np.concatenate([-results[c]["negmax_out"] for c in range(N_CORES)], axis=1)
    s = np.concatenate([results[c]["sum_out"] for c in range(N_CORES)], axis=1)
    tsum = sum(results[c]["tsum_out"].astype(np.float64).sum() for c in range(N_CORES))

    m64 = m.astype(np.float64)
    s64 = s.astype(np.float64)
    M = m64.max(axis=1, keepdims=True)                    # [B, 1]
    S = (s64 * np.exp(m64 - M)).sum(axis=1)               # [B]
    logZ = M[:, 0] + np.log(S)                            # [B]
    loss = logZ.mean() - tsum / (B * SEQ)
    return np.float32(loss)


def kernel(z, labels, W0, b0, W1, b1):
    nc = _get_nc()
    in_maps = _make_in_maps(z, labels, W0, b0, W1, b1)
    res = run_bass_kernel_spmd(nc, in_maps, list(range(N_CORES)))
    return _combine(res.results)
